# revision 17
# baseline (speedup 1.0000x reference)
"""Trainium2 Bass kernel for nn_CustomResidualAttentionBlock (open_clip-style block).

Sharding: head-parallel attention over 8 cores. Core c owns heads (2c, 2c+1)
for BOTH batches. Each core receives the FULL x of both batches (host-sharded,
free: [C, 4096] bf16), computes q/k/v for only its 2 heads over all tokens, and
runs attention fully locally -- no k/v AllGather at all. A single 8-core
AllToAll (128KB/shard, 1MB total) then redistributes O^T to token-sharding
(core c owns global token block c = batch c//4, tokens [512*(c%4):...]), after
which out-proj, the LN chains and the MLP run on the core's own 512 tokens,
identical to a sequence-parallel layout.

LayerNorm1 statistics are computed per-core on its OWN 512 tokens and shared
via a 32KB 8-core AllGather (latency-bound, hidden under the v projection).
The statistics enter the qkv projections as rank-1 corrections after the main
matmuls (y_ln = rstd*(raw - mu*colsum(W)) + b); for q/k the per-token rstd
cancels inside the l2-normalization (zero qkv bias), so q-hat/k-hat come
straight from the LN-corrected PSUM.

Host-side folds (exact math, fp32):
  - ln1_g into wqkT/wvT columns; ln2_g into wfcT; ls1 into ln_attn affine;
    ls2 into proj weights/bias
  - logit_scale (clamped+exp'd) into the q-norm ones-block (1/lsc^2 entries)
  - head_scale into the rowsum-replication lhsT (sqrt(hs) entries)
  - column sums of the (bf16) qkv weights for the LN rank-1 correction

All big matmuls run in bf16 with fp32 PSUM accumulation; layernorm statistics,
softmax row sums and normalization factors stay in fp32.
"""
import numpy as np
import ml_dtypes

import concourse.bass as bass
import concourse.mybir as mybir
import concourse.tile as tile
from concourse import bacc
from concourse.bass_utils import run_bass_kernel_spmd

F32 = mybir.dt.float32
BF16 = mybir.dt.bfloat16
BF_NP = ml_dtypes.bfloat16
AF = mybir.ActivationFunctionType
ALU = mybir.AluOpType

B, L, C, H = 2, 2048, 1024, 16
HD = C // H          # 64
MLP = 4 * C          # 4096
N_CORES = 8
T = (B * L) // N_CORES  # 512 own tokens per core
LA = B * L           # 4096 total tokens seen by every core
CT = C // 128        # 8 channel tiles
KM = L // 128        # 16 key chunks per batch
QC = L // 512        # 4 query chunks per batch
NB = LA // T         # 8 global token blocks
TC8 = LA // 512      # 8 projection column chunks
VKM = LA // 128      # 32 v token chunks
MT_FC = MLP // 128   # 32
LN_EPS = 1e-5
N_FC_PRE = 3
VP = 68              # padded per-(chunk,head) v stride (65 used, 68 for 16B align)

TRACE = False
TRACE_DIR = "/tmp/bass_trace"
LAST_EXEC_NS = None
LAST_RESULTS = None

_NC_CACHE = None


def _build():
    nc = bacc.Bacc(None, target_bir_lowering=False, debug=False, num_devices=N_CORES)

    # ---- I/O ----
    xtf_d = nc.dram_tensor("xTf", [C, T], F32, kind="ExternalInput")
    xtb_d = nc.dram_tensor("xTb", [C, LA], BF16, kind="ExternalInput")
    xtt_d = nc.dram_tensor("xTt", [LA, C], BF16, kind="ExternalInput")
    out_d = nc.dram_tensor("outT", [C, T], F32, kind="ExternalOutput")
    wqk_d = nc.dram_tensor("wqkT", [2, 128, CT, 128], BF16, kind="ExternalInput")
    wv_d = nc.dram_tensor("wvT", [C, 128], BF16, kind="ExternalInput")
    wo_d = nc.dram_tensor("woT", [C, C], BF16, kind="ExternalInput")
    wfc_d = nc.dram_tensor("wfcT", [MT_FC, 128, CT, 128], BF16, kind="ExternalInput")
    wpj_d = nc.dram_tensor("wprojT", [MLP, C], BF16, kind="ExternalInput")
    cqk_d = nc.dram_tensor("csumqk", [1, 256], BF16, kind="ExternalInput")
    cv_d = nc.dram_tensor("csumv", [1, 128], BF16, kind="ExternalInput")
    vb_d = nc.dram_tensor("vb", [1, 128], F32, kind="ExternalInput")
    outb_d = nc.dram_tensor("outb_s", [128, 8], F32, kind="ExternalInput")
    fcb_d = nc.dram_tensor("fcb", [128, 32], F32, kind="ExternalInput")
    pjb_d = nc.dram_tensor("projb_s", [128, 8], F32, kind="ExternalInput")
    ga_d = nc.dram_tensor("gattn_s", [128, 8], F32, kind="ExternalInput")
    ba_d = nc.dram_tensor("battn_s", [128, 8], F32, kind="ExternalInput")
    onesq_d = nc.dram_tensor("onesq", [128, 128], BF16, kind="ExternalInput")
    onesk_d = nc.dram_tensor("onesk", [128, 128], BF16, kind="ExternalInput")
    invhs_d = nc.dram_tensor("invhs", [1, 128], BF16, kind="ExternalInput")

    groups = [list(range(N_CORES))]

    with tile.TileContext(nc) as tc:
        with (
            tc.tile_pool(name="cn", bufs=1) as cn,
            tc.tile_pool(name="mid", bufs=1) as mid,
            tc.tile_pool(name="dram", bufs=1, space="DRAM") as dram,
        ):
            # ---- persistent activations (post-attention phases) ----
            xTf = [mid.tile([128, T], F32, name=f"xTf{c}") for c in range(CT)]
            uT = [mid.tile([128, T], F32, name=f"uT{c}") for c in range(CT)]
            h1T = [mid.tile([128, T], BF16, name=f"h1T{c}") for c in range(CT)]

            # ---- small constants ----
            eps_sb = cn.tile([128, 1], F32)
            nc.vector.memset(eps_sb[:], LN_EPS)
            ones_cb = cn.tile([128, 1], BF16)
            nc.vector.memset(ones_cb[:], 1.0)
            ones_r = cn.tile([1, 128], F32)
            nc.vector.memset(ones_r[:], 1.0)
            cqk_sb = cn.tile([1, 256], BF16)
            nc.sync.dma_start(out=cqk_sb[:], in_=cqk_d[:])
            cv_row = cn.tile([1, 128], BF16)
            nc.sync.dma_start(out=cv_row[:], in_=cv_d[:])
            onesq_sb = cn.tile([128, 128], BF16)
            nc.sync.dma_start(out=onesq_sb[:], in_=onesq_d[:])
            onesk_sb = cn.tile([128, 128], BF16)
            nc.sync.dma_start(out=onesk_sb[:], in_=onesk_d[:])
            invhs_sb = cn.tile([1, 128], BF16)
            nc.sync.dma_start(out=invhs_sb[:], in_=invhs_d[:])
            fcb_sb = cn.tile([128, 32], F32)
            nc.sync.dma_start(out=fcb_sb[:], in_=fcb_d[:])
            outb_sb = cn.tile([128, 8], F32)
            nc.sync.dma_start(out=outb_sb[:], in_=outb_d[:])
            pjb_sb = cn.tile([128, 8], F32)
            nc.sync.dma_start(out=pjb_sb[:], in_=pjb_d[:])
            ga_sb = cn.tile([128, 8], F32)
            nc.sync.dma_start(out=ga_sb[:], in_=ga_d[:])
            ba_sb = cn.tile([128, 8], F32)
            nc.sync.dma_start(out=ba_sb[:], in_=ba_d[:])
            vb_bc = cn.tile([128, 128], F32)
            nc.sync.dma_start(out=vb_bc[:], in_=vb_d[:].to_broadcast([128, 128]))

            # ---- collective buffers ----
            nm_d = dram.tile([VKM, 128], BF16)
            a2a_in = dram.tile([NB, 128, T], BF16)
            a2a_out = dram.tile([NB, 128, T], BF16)

            def ln_stats_T(tiles, rot, psp):
                """LN stats over the partition (channel) axis of 8 [128, T] tiles.

                Returns (a_rep, c_rep): rstd and mu*rstd replicated to 128
                partitions (fp32 sbuf).
                """
                pmean = psp.tile([1, T], F32, tag="stat", bufs=2, name="pmean")
                pvar = psp.tile([1, T], F32, tag="stat", bufs=2, name="pvar")
                if tiles[0].dtype == BF16:
                    btiles = [tiles[ct][:] for ct in range(CT)]
                else:
                    btiles = []
                    for ct in range(CT):
                        bt = rot.tile([128, T], BF16, tag="lnbt", name=f"lnbt{ct}")
                        nc.vector.tensor_copy(out=bt[:], in_=tiles[ct][:])
                        btiles.append(bt[:])
                for ct in range(CT):
                    nc.tensor.matmul(pmean[:], ones_cb[:], btiles[ct],
                                     start=(ct == 0), stop=(ct == CT - 1))
                for ct in range(CT):
                    sq = rot.tile([128, T], BF16, tag="lnsq", name=f"lnsq{ct}")
                    nc.scalar.activation(out=sq[:], in_=btiles[ct], func=AF.Square)
                    nc.tensor.matmul(pvar[:], ones_cb[:], sq[:],
                                     start=(ct == 0), stop=(ct == CT - 1))
                mu = rot.tile([1, T], F32, tag="lnmu", bufs=1, name="mu")
                nc.vector.tensor_scalar(out=mu[:], in0=pmean[:], scalar1=1.0 / C,
                                        scalar2=None, op0=ALU.mult)
                m2 = rot.tile([1, T], F32, tag="lnm2", bufs=1, name="m2")
                nc.vector.tensor_mul(out=m2[:], in0=mu[:], in1=mu[:])
                var = rot.tile([1, T], F32, tag="lnvar", bufs=1, name="var")
                # var = E[x^2] - mu^2
                nc.vector.scalar_tensor_tensor(
                    out=var[:], in0=pvar[:], scalar=1.0 / C, in1=m2[:],
                    op0=ALU.mult, op1=ALU.subtract,
                )
                rstd = rot.tile([1, T], F32, tag="lnrstd", bufs=1, name="rstd")
                nc.scalar.activation(out=rstd[:], in_=var[:],
                                     func=AF.Abs_reciprocal_sqrt,
                                     bias=eps_sb[0:1, :])
                murstd = rot.tile([1, T], F32, tag="lnmr", bufs=1, name="murstd")
                nc.vector.tensor_mul(out=murstd[:], in0=mu[:], in1=rstd[:])
                # replicate to 128 partitions via K=1 matmuls
                prep = psp.tile([128, T], F32, tag="repl", bufs=1, name="prep")
                a_rep = rot.tile([128, T], F32, tag="arep", bufs=1, name="a_rep")
                nc.tensor.matmul(prep[:], ones_r[:], rstd[:], start=True, stop=True)
                nc.vector.tensor_copy(out=a_rep[:], in_=prep[:])
                prep2 = psp.tile([128, T], F32, tag="repl", bufs=1, name="prep2")
                c_rep = rot.tile([128, T], F32, tag="crep", bufs=1, name="c_rep")
                nc.tensor.matmul(prep2[:], ones_r[:], murstd[:], start=True,
                                 stop=True)
                nc.vector.tensor_copy(out=c_rep[:], in_=prep2[:])
                return a_rep, c_rep

            with tc.tile_pool(name="ab", bufs=1) as ab:
                # q/k (l2-normed, scale-folded) and v for the 2 local heads
                qTn = ab.tile([128, LA], BF16, name="qTn")
                kTn = ab.tile([128, LA], BF16, name="kTn")
                vT = ab.tile([128, VKM, 2, VP], BF16, name="vT")
                OT = ab.tile([128, LA], BF16, name="OT")

                # ==== phase A: stats + qkv (head-local, both batches) ====
                with (
                    tc.tile_pool(name="pa", bufs=1) as pa,
                    tc.tile_pool(name="rot1", bufs=2) as rot1,
                    tc.tile_pool(name="ps1", bufs=1, space="PSUM") as ps1,
                ):
                    xTb = [pa.tile([128, LA], BF16, name=f"xTb{c}")
                           for c in range(CT)]
                    for ct in range(CT):
                        nc.sync.dma_start(out=xTb[ct][:],
                                          in_=xtb_d[128 * ct:128 * (ct + 1), :])
                    wv_sb = pa.tile([128, CT, 128], BF16)
                    nc.sync.dma_start(
                        out=wv_sb[:],
                        in_=wv_d[:].rearrange("(k p) m -> p k m", p=128),
                    )

                    # ---- LN1 stats: local bn_stats over token-major x ----
                    mvall = pa.tile([128, VKM, 2], F32, name="mvall")
                    for t in range(VKM):
                        xtt_t = rot1.tile([128, C], BF16, tag="xtt", bufs=8,
                                          name=f"xtt{t}")
                        eng = [nc.gpsimd, nc.scalar][t % 2]
                        eng.dma_start(
                            out=xtt_t[:], in_=xtt_d[128 * t:128 * (t + 1), :]
                        )
                        bstat = rot1.tile([128, 2, 6], F32, tag="bst", bufs=2,
                                          name=f"bst{t}")
                        xr = xtt_t[:].rearrange("p (s f) -> p s f", f=512)
                        for sub in range(2):
                            nc.vector.bn_stats(out=bstat[:, sub, :],
                                               in_=xr[:, sub, :])
                        nc.vector.bn_aggr(out=mvall[:, t, :], in_=bstat[:])
                    # negmu / rstd columns (token-major [128, VKM])
                    negmu_cols = pa.tile([128, VKM], BF16, name="negmu_cols")
                    nc.vector.tensor_scalar(out=negmu_cols[:],
                                            in0=mvall[:, :, 0:1],
                                            scalar1=-1.0, scalar2=None,
                                            op0=ALU.mult)
                    sdev = rot1.tile([128, VKM], F32, tag="sdev", bufs=1,
                                     name="sdev")
                    nc.scalar.activation(out=sdev[:], in_=mvall[:, :, 1:2],
                                         func=AF.Sqrt, bias=eps_sb[:])
                    stat_cols = pa.tile([128, VKM], BF16, name="stat_cols")
                    with nc.allow_low_precision(reason="bf16 rstd is ample"):
                        nc.vector.reciprocal(out=stat_cols[:], in_=sdev[:])
                    # negmu as a [1, LA] row via a DRAM transpose bounce
                    nc.scalar.dma_start(
                        out=nm_d[:].rearrange("c p -> p c"), in_=negmu_cols[:]
                    )
                    negmu_all = pa.tile([1, LA], BF16)
                    nc.scalar.dma_start(out=negmu_all[:], in_=nm_d[:])

                    # ---- v (token-major: partitions = tokens) ----
                    nc.vector.memset(vT[:, :, :, HD:HD + 1], 1.0)
                    for t in range(VKM):
                        pv = ps1.tile([128, 128], F32, tag="mm", bufs=3,
                                      name=f"pv{t}")
                        nc.tensor.matmul(
                            pv[:], negmu_all[0:1, 128 * t:128 * (t + 1)],
                            cv_row[:],
                            start=True, stop=False,
                        )
                        for kc in range(CT):
                            nc.tensor.matmul(
                                pv[:], xTb[kc][:, 128 * t:128 * (t + 1)],
                                wv_sb[:, kc, :],
                                start=False, stop=(kc == CT - 1),
                            )
                        # v = rstd*(raw - mu*csumv) + vb
                        nc.vector.scalar_tensor_tensor(
                            out=vT[:, t, :, 0:HD], in0=pv[:],
                            scalar=stat_cols[:, t:t + 1], in1=vb_bc[:],
                            op0=ALU.mult, op1=ALU.add,
                        )

                    # ---- q,k rows (2 row-tiles x 8 token chunks) ----
                    for mt in range(2):
                        wqk_t = rot1.tile([128, CT, 128], BF16, tag="wqk",
                                          bufs=2, name=f"wqk{mt}")
                        nc.sync.dma_start(out=wqk_t[:], in_=wqk_d[mt])
                        for tc8 in range(TC8):
                            sl = slice(512 * tc8, 512 * (tc8 + 1))
                            pqk = ps1.tile([128, 512], F32, tag="mm", bufs=3,
                                           name=f"pqk{mt}_{tc8}")
                            # fold the -mu*colsum(W) LN correction into the accum
                            nc.tensor.matmul(
                                pqk[:], cqk_sb[0:1, 128 * mt:128 * (mt + 1)],
                                negmu_all[0:1, sl],
                                start=True, stop=False,
                            )
                            for kc in range(CT):
                                nc.tensor.matmul(
                                    pqk[:], wqk_t[:, kc, :], xTb[kc][:, sl],
                                    start=False, stop=(kc == CT - 1),
                                )
                            # zero qkv bias: rstd cancels inside l2-norm
                            sq = rot1.tile([128, 512], BF16, tag="sq", bufs=3,
                                           name=f"sq{mt}_{tc8}")
                            nc.scalar.activation(out=sq[:], in_=pqk[:],
                                                 func=AF.Square)
                            pn = ps1.tile([128, 512], F32, tag="nrm", bufs=2,
                                          name=f"pn{mt}_{tc8}")
                            ones = onesq_sb[:] if mt == 0 else onesk_sb[:]
                            nc.tensor.matmul(pn[:], ones, sq[:], start=True,
                                             stop=True)
                            rrec = rot1.tile([128, 512], F32, tag="rrec",
                                             name=f"rrec{mt}_{tc8}")
                            nc.scalar.activation(out=rrec[:], in_=pn[:],
                                                 func=AF.Abs_reciprocal_sqrt)
                            dst = qTn if mt == 0 else kTn
                            nc.vector.tensor_mul(out=dst[:, sl], in0=pqk[:],
                                                 in1=rrec[:])

                    # prefetch (data-ready at issue): residual xTf
                    for ct in range(CT):
                        nc.sync.dma_start(out=xTf[ct][:],
                                          in_=xtf_d[128 * ct:128 * (ct + 1), :])

                # ==== phase B: attention (2 local heads, both batches) ====
                with (
                    tc.tile_pool(name="rot2", bufs=2) as rot2,
                    tc.tile_pool(name="ps2", bufs=1, space="PSUM") as ps2,
                ):
                    for bb in range(B):
                        for qc in range(QC):
                            blk = QC * bb + qc
                            qsl = slice(L * bb + 512 * qc, L * bb + 512 * (qc + 1))
                            po = [
                                ps2.tile([HD + 1, T], F32, tag=f"o{h2}", bufs=1,
                                         name=f"po{blk}_{h2}")
                                for h2 in range(2)
                            ]
                            pTs = []
                            for kb in range(KM // 2):
                                psS = ps2.tile([128, 4, T], F32, tag="s", bufs=1,
                                               name=f"psS{blk}_{kb}")
                                for kl in range(2):
                                    km = 2 * kb + kl
                                    ksl = slice(L * bb + 128 * km,
                                                L * bb + 128 * (km + 1))
                                    for h2 in range(2):
                                        nc.tensor.matmul(
                                            psS[:, 2 * kl + h2, :],
                                            kTn[64 * h2:64 * (h2 + 1), ksl],
                                            qTn[64 * h2:64 * (h2 + 1), qsl],
                                            start=True, stop=True,
                                        )
                                pT = rot2.tile([128, 4, T], BF16, tag="pT",
                                               bufs=8, name=f"pT{blk}_{kb}")
                                nc.scalar.activation(out=pT[:], in_=psS[:],
                                                     func=AF.Exp)
                                pTs.append(pT)
                                # software pipeline: PV of the previous pair
                                # issues after this pair's S, so the tensor
                                # queue never stalls behind the exp
                                if kb > 0:
                                    for kl in range(2):
                                        km = 2 * (kb - 1) + kl
                                        for h2 in range(2):
                                            nc.tensor.matmul(
                                                po[h2][:],
                                                vT[:, KM * bb + km, h2, 0:HD + 1],
                                                pTs[kb - 1][:, 2 * kl + h2, :],
                                                start=(kb == 1 and kl == 0),
                                                stop=False,
                                            )
                            for kl in range(2):
                                km = KM - 2 + kl
                                for h2 in range(2):
                                    nc.tensor.matmul(
                                        po[h2][:],
                                        vT[:, KM * bb + km, h2, 0:HD + 1],
                                        pTs[KM // 2 - 1][:, 2 * kl + h2, :],
                                        start=False,
                                        stop=(kl == 1),
                                    )
                            for h2 in range(2):
                                # o = po * hs/rowsum  (vector recip + K=1 repl)
                                rcp = rot2.tile([1, T], BF16, tag="rcp")
                                with nc.allow_low_precision(
                                        reason="bf16 softmax rowsum recip"):
                                    nc.vector.reciprocal(
                                        out=rcp[:], in_=po[h2][HD:HD + 1, :])
                                prr = ps2.tile([HD, T], F32, tag="prr", bufs=2,
                                               name=f"prr{h2}")
                                nc.tensor.matmul(
                                    prr[:],
                                    invhs_sb[0:1, HD * h2:HD * (h2 + 1)], rcp[:],
                                    start=True, stop=True,
                                )
                                rep = rot2.tile([HD, T], F32, tag="orec")
                                nc.vector.tensor_copy(out=rep[:], in_=prr[:])
                                nc.vector.tensor_mul(
                                    out=OT[64 * h2:64 * (h2 + 1), qsl],
                                    in0=po[h2][0:HD, :],
                                    in1=rep[:],
                                )
                            nc.gpsimd.dma_start(
                                out=a2a_in[blk], in_=OT[:, qsl]
                            )
                    nc.gpsimd.collective_compute(
                        "AllToAll", ALU.bypass, replica_groups=groups,
                        ins=[a2a_in.opt()], outs=[a2a_out.opt()],
                    )

            # ==== phase C: out projection (transposed out) + ln_attn + ln2 ====
            with (
                tc.tile_pool(name="pc", bufs=1) as pc,
                tc.tile_pool(name="rot3", bufs=2) as rot3,
                tc.tile_pool(name="ps3", bufs=1, space="PSUM") as ps3,
            ):
                wo_sb = pc.tile([128, CT, C], BF16)
                nc.sync.dma_start(
                    out=wo_sb[:],
                    in_=wo_d[:].rearrange("(k p) m -> p k m", p=128),
                )
                oa = [pc.tile([128, T], BF16, name=f"oa{k}") for k in range(CT)]
                for kc in range(CT):
                    nc.scalar.dma_start(out=oa[kc][:], in_=a2a_out[kc])
                yT = [rot3.tile([128, T], F32, tag="yT", bufs=8, name=f"yT{m}")
                      for m in range(CT)]
                for m in range(CT):
                    py = ps3.tile([128, T], F32, tag="y", bufs=3, name=f"py{m}")
                    for kc in range(CT):
                        nc.tensor.matmul(
                            py[:],
                            wo_sb[:, kc, 128 * m:128 * (m + 1)],
                            oa[kc][:],
                            start=(kc == 0), stop=(kc == CT - 1),
                        )
                    nc.vector.tensor_scalar(out=yT[m][:], in0=py[:],
                                            scalar1=outb_sb[:, m:m + 1],
                                            scalar2=None, op0=ALU.add)
                a2_rep, c2_rep = ln_stats_T(yT, rot3, ps3)
                # uT = xT + (yT - mu)*rstd*g' + b'   (g', b' have ls1 folded)
                for ct in range(CT):
                    t1 = rot3.tile([128, T], F32, tag="ut1", name=f"ut1_{ct}")
                    nc.gpsimd.tensor_mul(out=t1[:], in0=yT[ct][:], in1=a2_rep[:])
                    t2 = rot3.tile([128, T], F32, tag="ut2", name=f"ut2_{ct}")
                    nc.gpsimd.tensor_sub(out=t2[:], in0=t1[:], in1=c2_rep[:])
                    t3 = rot3.tile([128, T], F32, tag="ut3", name=f"ut3_{ct}")
                    nc.vector.tensor_scalar(out=t3[:], in0=t2[:],
                                            scalar1=ga_sb[:, ct:ct + 1],
                                            scalar2=ba_sb[:, ct:ct + 1],
                                            op0=ALU.mult, op1=ALU.add)
                    nc.vector.tensor_add(out=uT[ct][:], in0=t3[:], in1=xTf[ct][:])
                a3_rep, c3_rep = ln_stats_T(uT, rot3, ps3)
                for ct in range(CT):
                    # h1T = (uT)*rstd - mu*rstd  (ln2 affine folded into fc)
                    t4 = rot3.tile([128, T], F32, tag="ut4", name=f"ut4_{ct}")
                    eng = nc.gpsimd if ct % 2 == 0 else nc.vector
                    eng.tensor_mul(out=t4[:], in0=uT[ct][:], in1=a3_rep[:])
                    eng.tensor_sub(out=h1T[ct][:], in0=t4[:], in1=c3_rep[:])

            # ---- phase D: fc + gelu; proj (transposed) + residual ----
            with (
                tc.tile_pool(name="pd", bufs=1) as pd,
                tc.tile_pool(name="rot4", bufs=2) as rot4,
                tc.tile_pool(name="ps4", bufs=1, space="PSUM") as ps4,
            ):
                gT = [pd.tile([128, T], BF16, name=f"gT{m}") for m in range(MT_FC)]
                for mt in range(MT_FC):
                    wfc_t = rot4.tile([128, CT, 128], BF16, tag="wfc", bufs=6,
                                      name=f"wfc{mt}")
                    nc.sync.dma_start(out=wfc_t[:], in_=wfc_d[mt])
                    pfc = ps4.tile([128, T], F32, tag="fc", bufs=3, name=f"pfc{mt}")
                    for kc in range(CT):
                        nc.tensor.matmul(
                            pfc[:], wfc_t[:, kc, :], h1T[kc][:],
                            start=(kc == 0), stop=(kc == CT - 1),
                        )
                    nc.scalar.activation(out=gT[mt][:], in_=pfc[:], func=AF.Gelu,
                                         bias=fcb_sb[:, mt:mt + 1])

                for half in range(2):
                    ppj = [
                        ps4.tile([128, T], F32, tag=f"pj{i}", bufs=1,
                                 name=f"ppj{i}")
                        for i in range(4)
                    ]
                    for mt in range(MT_FC):
                        wpj_t = rot4.tile([128, 4, 128], BF16, tag="wpj", bufs=8,
                                          name=f"wpj{half}_{mt}")
                        nc.sync.dma_start(
                            out=wpj_t[:],
                            in_=wpj_d[128 * mt:128 * (mt + 1),
                                      512 * half:512 * (half + 1)].rearrange(
                                "p (i m) -> p i m", m=128
                            ),
                        )
                        for i in range(4):
                            nc.tensor.matmul(
                                ppj[i][:],
                                wpj_t[:, i, :],
                                gT[mt][:],
                                start=(mt == 0), stop=(mt == MT_FC - 1),
                            )
                    for i in range(4):
                        ct = 4 * half + i
                        o1 = rot4.tile([128, T], F32, tag="ofin",
                                       name=f"of{half}_{i}")
                        nc.vector.tensor_scalar(out=o1[:], in0=ppj[i][:],
                                                scalar1=pjb_sb[:, ct:ct + 1],
                                                scalar2=None, op0=ALU.add)
                        nc.vector.tensor_add(out=o1[:], in0=o1[:], in1=uT[ct][:])
                        nc.gpsimd.dma_start(
                            out=out_d[128 * ct:128 * (ct + 1), :], in_=o1[:]
                        )

    nc.compile()
    return nc


def _host_prep(inp):
    """Shared + per-core (head-pair) folded weights."""
    f32 = np.float32
    ln1_g = np.asarray(inp["ln1_g"], f32)
    ln1_b = np.asarray(inp["ln1_b"], f32)
    ln2_g = np.asarray(inp["ln2_g"], f32)
    ln2_b = np.asarray(inp["ln2_b"], f32)
    in_w = np.asarray(inp["in_proj_w"], f32)
    in_b = np.asarray(inp["in_proj_b"], f32)
    fc_w = np.asarray(inp["fc_w"], f32)
    proj_w = np.asarray(inp["proj_w"], f32)
    ls1 = np.asarray(inp["ls1"], f32)
    ls2 = np.asarray(inp["ls2"], f32)

    # fast path requires zero qkv bias (rstd cancels inside the l2-norm)
    qk_bias = in_b[:2 * C] + ln1_b @ in_w[:2 * C].T
    assert np.abs(qk_bias).max() < 1e-6, "nonzero qk bias: fast path invalid"

    s = {}
    s["woT"] = np.ascontiguousarray(np.asarray(inp["out_w"], f32).T).astype(BF_NP)
    s["outb_s"] = np.ascontiguousarray(
        np.asarray(inp["out_b"], f32).reshape(8, 128).T
    )
    wfcT = np.ascontiguousarray((fc_w * ln2_g[None, :]).T).astype(BF_NP)
    s["wfcT"] = np.ascontiguousarray(
        wfcT.reshape(CT, 128, MT_FC, 128).transpose(2, 1, 0, 3)
    )
    fcb = np.asarray(inp["fc_b"], f32) + ln2_b @ fc_w.T
    s["fcb"] = np.ascontiguousarray(fcb.reshape(32, 128).T).astype(f32)
    s["wprojT"] = np.ascontiguousarray((proj_w * ls2[:, None]).T).astype(BF_NP)
    s["projb_s"] = np.ascontiguousarray(
        (ls2 * np.asarray(inp["proj_b"], f32)).reshape(8, 128).T
    )
    s["gattn_s"] = np.ascontiguousarray(
        (ls1 * np.asarray(inp["ln_attn_g"], f32)).reshape(8, 128).T
    )
    s["battn_s"] = np.ascontiguousarray(
        (ls1 * np.asarray(inp["ln_attn_b"], f32)).reshape(8, 128).T
    )
    onesk = np.zeros((128, 128), f32)
    onesk[:64, :64] = 1.0
    onesk[64:, 64:] = 1.0
    s["onesk"] = onesk.astype(BF_NP)

    lsc = np.exp(np.minimum(np.asarray(inp["logit_scale"], f32).reshape(H),
                            np.log(100.0)))
    hs = np.asarray(inp["head_scale"], f32)

    per_c = []
    for c in range(N_CORES):
        p = {}
        rq = slice(128 * c, 128 * (c + 1))
        rk = slice(C + 128 * c, C + 128 * (c + 1))
        rv = slice(2 * C + 128 * c, 2 * C + 128 * (c + 1))
        w_qk = np.concatenate([in_w[rq], in_w[rk]], 0)        # [256, C]
        wqkT = np.ascontiguousarray((w_qk * ln1_g[None, :]).T).astype(BF_NP)
        p["wqkT"] = np.ascontiguousarray(
            wqkT.reshape(CT, 128, 2, 128).transpose(2, 1, 0, 3)
        )
        p["csumqk"] = wqkT.astype(f32).sum(0).reshape(1, 256).astype(BF_NP)
        w_v = in_w[rv]
        wvT = np.ascontiguousarray((w_v * ln1_g[None, :]).T).astype(BF_NP)
        p["wvT"] = wvT
        p["csumv"] = wvT.astype(f32).sum(0).reshape(1, 128).astype(BF_NP)
        p["vb"] = (in_b[rv] + ln1_b @ w_v.T).reshape(1, 128).astype(f32)
        onesq = np.zeros((128, 128), f32)
        for h2 in range(2):
            h = 2 * c + h2
            onesq[64 * h2:64 * (h2 + 1),
                  64 * h2:64 * (h2 + 1)] = 1.0 / lsc[h] ** 2
        p["onesq"] = onesq.astype(BF_NP)
        invhs = np.zeros((1, 128), f32)
        for h2 in range(2):
            invhs[0, HD * h2:HD * (h2 + 1)] = hs[2 * c + h2]
        p["invhs"] = invhs.astype(BF_NP)
        per_c.append(p)
    return s, per_c


def kernel(**inputs) -> np.ndarray:
    global _NC_CACHE, LAST_EXEC_NS, LAST_RESULTS
    if _NC_CACHE is None:
        _NC_CACHE = _build()
    nc = _NC_CACHE

    shared, per_c = _host_prep(inputs)
    x = np.asarray(inputs["x"], np.float32)
    # both batches, channel-major: [C, 4096] = [x0^T | x1^T]
    xT = np.ascontiguousarray(
        np.concatenate([x[b].T for b in range(B)], axis=1)
    )
    xTb = xT.astype(BF_NP)
    xTt = np.ascontiguousarray(xTb.T)                          # [4096, C]

    in_maps = []
    for c in range(N_CORES):
        m = dict(shared)
        m.update(per_c[c])
        m["xTb"] = xTb
        m["xTt"] = xTt
        m["xTf"] = np.ascontiguousarray(xT[:, T * c:T * (c + 1)])
        in_maps.append(m)

    kwargs = {}
    if TRACE:
        import os
        os.makedirs(TRACE_DIR, exist_ok=True)
        kwargs = dict(trace=True, tmpdir=TRACE_DIR)
    res = run_bass_kernel_spmd(nc, in_maps, list(range(N_CORES)), **kwargs)
    LAST_EXEC_NS = res.exec_time_ns
    LAST_RESULTS = res
    out = np.zeros((B, L, C), np.float32)
    for c in range(N_CORES):
        b, r = c // 4, c % 4
        out[b, L // 4 * r:L // 4 * (r + 1), :] = res.results[c]["outT"].T
    return out


# revision 18
# speedup vs baseline: 1.1098x; 1.1098x over previous
"""Trainium2 Bass kernel for nn_CustomResidualAttentionBlock (open_clip-style block).

Sharding: head-parallel attention over 8 cores. Core c owns heads (2c, 2c+1)
for BOTH batches. Each core receives the FULL x of both batches (host-sharded,
free: [C, 4096] bf16), computes q/k/v for only its 2 heads over all tokens, and
runs attention fully locally -- no k/v AllGather at all. A single 8-core
AllToAll (128KB/shard, 1MB total) then redistributes O^T to token-sharding
(core c owns global token block c = batch c//4, tokens [512*(c%4):...]), after
which out-proj, the LN chains and the MLP run on the core's own 512 tokens,
identical to a sequence-parallel layout.

LayerNorm1 statistics are computed per-core on its OWN 512 tokens and shared
via a 32KB 8-core AllGather (latency-bound, hidden under the v projection).
The statistics enter the qkv projections as rank-1 corrections after the main
matmuls (y_ln = rstd*(raw - mu*colsum(W)) + b); for q/k the per-token rstd
cancels inside the l2-normalization (zero qkv bias), so q-hat/k-hat come
straight from the LN-corrected PSUM.

Host-side folds (exact math, fp32):
  - ln1_g into wqkT/wvT columns; ln2_g into wfcT; ls1 into ln_attn affine;
    ls2 into proj weights/bias
  - logit_scale (clamped+exp'd) into the q-norm ones-block (1/lsc^2 entries)
  - head_scale into the rowsum-replication lhsT (sqrt(hs) entries)
  - column sums of the (bf16) qkv weights for the LN rank-1 correction

All big matmuls run in bf16 with fp32 PSUM accumulation; layernorm statistics,
softmax row sums and normalization factors stay in fp32.
"""
import numpy as np
import ml_dtypes

import concourse.bass as bass
import concourse.mybir as mybir
import concourse.tile as tile
from concourse import bacc
from concourse.bass_utils import run_bass_kernel_spmd

F32 = mybir.dt.float32
BF16 = mybir.dt.bfloat16
FP8 = mybir.dt.float8e4
PM_DR = mybir.MatmulPerfMode.DoubleRow
BF_NP = ml_dtypes.bfloat16
F8_NP = ml_dtypes.float8_e4m3
FC_SCALE = 64.0
PJ_SCALE = 32.0
AF = mybir.ActivationFunctionType
ALU = mybir.AluOpType

B, L, C, H = 2, 2048, 1024, 16
HD = C // H          # 64
MLP = 4 * C          # 4096
N_CORES = 8
T = (B * L) // N_CORES  # 512 own tokens per core
LA = B * L           # 4096 total tokens seen by every core
CT = C // 128        # 8 channel tiles
KM = L // 128        # 16 key chunks per batch
QC = L // 512        # 4 query chunks per batch
NB = LA // T         # 8 global token blocks
TC8 = LA // 512      # 8 projection column chunks
VKM = LA // 128      # 32 v token chunks
MT_FC = MLP // 128   # 32
LN_EPS = 1e-5
N_FC_PRE = 3
VP = 68              # padded per-(chunk,head) v stride (65 used, 68 for 16B align)

TRACE = False
TRACE_DIR = "/tmp/bass_trace"
LAST_EXEC_NS = None
LAST_RESULTS = None

_NC_CACHE = None


def _build():
    nc = bacc.Bacc(None, target_bir_lowering=False, debug=False, num_devices=N_CORES)

    # ---- I/O ----
    xtf_d = nc.dram_tensor("xTf", [C, T], F32, kind="ExternalInput")
    xtb_d = nc.dram_tensor("xTb", [C, LA], BF16, kind="ExternalInput")
    xtt_d = nc.dram_tensor("xTt", [LA, C], BF16, kind="ExternalInput")
    out_d = nc.dram_tensor("outT", [C, T], F32, kind="ExternalOutput")
    wqk_d = nc.dram_tensor("wqkT", [2, 128, CT, 128], BF16, kind="ExternalInput")
    wv_d = nc.dram_tensor("wvT", [C, 128], BF16, kind="ExternalInput")
    wo_d = nc.dram_tensor("woT", [C, C], BF16, kind="ExternalInput")
    wfc_d = nc.dram_tensor("wfcT", [MT_FC, 128, 4, 2, 128], FP8,
                           kind="ExternalInput")
    wpj_d = nc.dram_tensor("wprojT", [16, 128, 2, C], FP8, kind="ExternalInput")
    ls2_d = nc.dram_tensor("ls2_s", [128, 8], F32, kind="ExternalInput")
    cqk_d = nc.dram_tensor("csumqk", [1, 256], BF16, kind="ExternalInput")
    cv_d = nc.dram_tensor("csumv", [1, 128], BF16, kind="ExternalInput")
    vb_d = nc.dram_tensor("vb", [1, 128], F32, kind="ExternalInput")
    outb_d = nc.dram_tensor("outb_s", [128, 8], F32, kind="ExternalInput")
    fcb_d = nc.dram_tensor("fcb", [128, 32], F32, kind="ExternalInput")
    pjb_d = nc.dram_tensor("projb_s", [128, 8], F32, kind="ExternalInput")
    ga_d = nc.dram_tensor("gattn_s", [128, 8], F32, kind="ExternalInput")
    ba_d = nc.dram_tensor("battn_s", [128, 8], F32, kind="ExternalInput")
    onesq_d = nc.dram_tensor("onesq", [128, 128], BF16, kind="ExternalInput")
    onesk_d = nc.dram_tensor("onesk", [128, 128], BF16, kind="ExternalInput")
    invhs_d = nc.dram_tensor("invhs", [1, 128], BF16, kind="ExternalInput")

    groups = [list(range(N_CORES))]

    with tile.TileContext(nc) as tc:
        with (
            tc.tile_pool(name="cn", bufs=1) as cn,
            tc.tile_pool(name="mid", bufs=1) as mid,
            tc.tile_pool(name="dram", bufs=1, space="DRAM") as dram,
        ):
            # ---- persistent activations (post-attention phases) ----
            xTf = [mid.tile([128, T], F32, name=f"xTf{c}") for c in range(CT)]
            uT = [mid.tile([128, T], F32, name=f"uT{c}") for c in range(CT)]
            h1T = mid.tile([128, CT, T], FP8, name="h1T")

            # ---- small constants ----
            eps_sb = cn.tile([128, 1], F32)
            nc.vector.memset(eps_sb[:], LN_EPS)
            ones_cb = cn.tile([128, 1], BF16)
            nc.vector.memset(ones_cb[:], 1.0)
            ones_r = cn.tile([1, 128], F32)
            nc.vector.memset(ones_r[:], 1.0)
            cqk_sb = cn.tile([1, 256], BF16)
            nc.sync.dma_start(out=cqk_sb[:], in_=cqk_d[:])
            cv_row = cn.tile([1, 128], BF16)
            nc.sync.dma_start(out=cv_row[:], in_=cv_d[:])
            onesq_sb = cn.tile([128, 128], BF16)
            nc.sync.dma_start(out=onesq_sb[:], in_=onesq_d[:])
            onesk_sb = cn.tile([128, 128], BF16)
            nc.sync.dma_start(out=onesk_sb[:], in_=onesk_d[:])
            invhs_sb = cn.tile([1, 128], BF16)
            nc.sync.dma_start(out=invhs_sb[:], in_=invhs_d[:])
            fcb_sb = cn.tile([128, 32], F32)
            nc.sync.dma_start(out=fcb_sb[:], in_=fcb_d[:])
            outb_sb = cn.tile([128, 8], F32)
            nc.sync.dma_start(out=outb_sb[:], in_=outb_d[:])
            pjb_sb = cn.tile([128, 8], F32)
            nc.sync.dma_start(out=pjb_sb[:], in_=pjb_d[:])
            ga_sb = cn.tile([128, 8], F32)
            nc.sync.dma_start(out=ga_sb[:], in_=ga_d[:])
            ba_sb = cn.tile([128, 8], F32)
            nc.sync.dma_start(out=ba_sb[:], in_=ba_d[:])
            ls2_sb = cn.tile([128, 8], F32)
            nc.sync.dma_start(out=ls2_sb[:], in_=ls2_d[:])
            vb_bc = cn.tile([128, 128], F32)
            nc.sync.dma_start(out=vb_bc[:], in_=vb_d[:].to_broadcast([128, 128]))

            # ---- collective buffers ----
            nm_d = dram.tile([VKM, 128], BF16)
            a2a_in = dram.tile([NB, 128, T], BF16)
            a2a_out = dram.tile([NB, 128, T], BF16)

            def ln_stats_T(tiles, rot, psp):
                """LN stats over the partition (channel) axis of 8 [128, T] tiles.

                Returns (a_rep, c_rep): rstd and mu*rstd replicated to 128
                partitions (fp32 sbuf).
                """
                pmean = psp.tile([1, T], F32, tag="stat", bufs=2, name="pmean")
                pvar = psp.tile([1, T], F32, tag="stat", bufs=2, name="pvar")
                if tiles[0].dtype == BF16:
                    btiles = [tiles[ct][:] for ct in range(CT)]
                else:
                    btiles = []
                    for ct in range(CT):
                        bt = rot.tile([128, T], BF16, tag="lnbt", name=f"lnbt{ct}")
                        nc.vector.tensor_copy(out=bt[:], in_=tiles[ct][:])
                        btiles.append(bt[:])
                for ct in range(CT):
                    nc.tensor.matmul(pmean[:], ones_cb[:], btiles[ct],
                                     start=(ct == 0), stop=(ct == CT - 1))
                for ct in range(CT):
                    sq = rot.tile([128, T], BF16, tag="lnsq", name=f"lnsq{ct}")
                    nc.scalar.activation(out=sq[:], in_=btiles[ct], func=AF.Square)
                    nc.tensor.matmul(pvar[:], ones_cb[:], sq[:],
                                     start=(ct == 0), stop=(ct == CT - 1))
                mu = rot.tile([1, T], F32, tag="lnmu", bufs=1, name="mu")
                nc.vector.tensor_scalar(out=mu[:], in0=pmean[:], scalar1=1.0 / C,
                                        scalar2=None, op0=ALU.mult)
                m2 = rot.tile([1, T], F32, tag="lnm2", bufs=1, name="m2")
                nc.vector.tensor_mul(out=m2[:], in0=mu[:], in1=mu[:])
                var = rot.tile([1, T], F32, tag="lnvar", bufs=1, name="var")
                # var = E[x^2] - mu^2
                nc.vector.scalar_tensor_tensor(
                    out=var[:], in0=pvar[:], scalar=1.0 / C, in1=m2[:],
                    op0=ALU.mult, op1=ALU.subtract,
                )
                rstd = rot.tile([1, T], F32, tag="lnrstd", bufs=1, name="rstd")
                nc.scalar.activation(out=rstd[:], in_=var[:],
                                     func=AF.Abs_reciprocal_sqrt,
                                     bias=eps_sb[0:1, :])
                murstd = rot.tile([1, T], F32, tag="lnmr", bufs=1, name="murstd")
                nc.vector.tensor_mul(out=murstd[:], in0=mu[:], in1=rstd[:])
                # replicate to 128 partitions via K=1 matmuls
                prep = psp.tile([128, T], F32, tag="repl", bufs=1, name="prep")
                a_rep = rot.tile([128, T], F32, tag="arep", bufs=1, name="a_rep")
                nc.tensor.matmul(prep[:], ones_r[:], rstd[:], start=True, stop=True)
                nc.vector.tensor_copy(out=a_rep[:], in_=prep[:])
                prep2 = psp.tile([128, T], F32, tag="repl", bufs=1, name="prep2")
                c_rep = rot.tile([128, T], F32, tag="crep", bufs=1, name="c_rep")
                nc.tensor.matmul(prep2[:], ones_r[:], murstd[:], start=True,
                                 stop=True)
                nc.vector.tensor_copy(out=c_rep[:], in_=prep2[:])
                return a_rep, c_rep

            with tc.tile_pool(name="ab", bufs=1) as ab:
                # q/k (l2-normed, scale-folded) and v for the 2 local heads
                qTn = ab.tile([128, LA], BF16, name="qTn")
                kTn = ab.tile([128, LA], BF16, name="kTn")
                vT = ab.tile([128, VKM, 2, VP], BF16, name="vT")
                OT = ab.tile([128, LA], BF16, name="OT")

                # ==== phase A: stats + qkv (head-local, both batches) ====
                with (
                    tc.tile_pool(name="pa", bufs=1) as pa,
                    tc.tile_pool(name="rot1", bufs=2) as rot1,
                    tc.tile_pool(name="ps1", bufs=1, space="PSUM") as ps1,
                ):
                    xTb = [pa.tile([128, LA], BF16, name=f"xTb{c}")
                           for c in range(CT)]
                    for ct in range(CT):
                        nc.sync.dma_start(out=xTb[ct][:],
                                          in_=xtb_d[128 * ct:128 * (ct + 1), :])
                    wv_sb = pa.tile([128, CT, 128], BF16)
                    nc.sync.dma_start(
                        out=wv_sb[:],
                        in_=wv_d[:].rearrange("(k p) m -> p k m", p=128),
                    )

                    # ---- LN1 stats: local bn_stats over token-major x ----
                    mvall = pa.tile([128, VKM, 2], F32, name="mvall")
                    for t in range(VKM):
                        xtt_t = rot1.tile([128, C], BF16, tag="xtt", bufs=8,
                                          name=f"xtt{t}")
                        eng = [nc.gpsimd, nc.scalar][t % 2]
                        eng.dma_start(
                            out=xtt_t[:], in_=xtt_d[128 * t:128 * (t + 1), :]
                        )
                        bstat = rot1.tile([128, 2, 6], F32, tag="bst", bufs=2,
                                          name=f"bst{t}")
                        xr = xtt_t[:].rearrange("p (s f) -> p s f", f=512)
                        for sub in range(2):
                            nc.vector.bn_stats(out=bstat[:, sub, :],
                                               in_=xr[:, sub, :])
                        nc.vector.bn_aggr(out=mvall[:, t, :], in_=bstat[:])
                    # negmu / rstd columns (token-major [128, VKM])
                    negmu_cols = pa.tile([128, VKM], BF16, name="negmu_cols")
                    nc.vector.tensor_scalar(out=negmu_cols[:],
                                            in0=mvall[:, :, 0:1],
                                            scalar1=-1.0, scalar2=None,
                                            op0=ALU.mult)
                    sdev = rot1.tile([128, VKM], F32, tag="sdev", bufs=1,
                                     name="sdev")
                    nc.scalar.activation(out=sdev[:], in_=mvall[:, :, 1:2],
                                         func=AF.Sqrt, bias=eps_sb[:])
                    stat_cols = pa.tile([128, VKM], BF16, name="stat_cols")
                    with nc.allow_low_precision(reason="bf16 rstd is ample"):
                        nc.vector.reciprocal(out=stat_cols[:], in_=sdev[:])
                    # negmu as a [1, LA] row via a DRAM transpose bounce
                    nc.scalar.dma_start(
                        out=nm_d[:].rearrange("c p -> p c"), in_=negmu_cols[:]
                    )
                    negmu_all = pa.tile([1, LA], BF16)
                    nc.scalar.dma_start(out=negmu_all[:], in_=nm_d[:])

                    # ---- v (token-major: partitions = tokens) ----
                    nc.vector.memset(vT[:, :, :, HD:HD + 1], 1.0)
                    for t in range(VKM):
                        pv = ps1.tile([128, 128], F32, tag="mm", bufs=3,
                                      name=f"pv{t}")
                        nc.tensor.matmul(
                            pv[:], negmu_all[0:1, 128 * t:128 * (t + 1)],
                            cv_row[:],
                            start=True, stop=False,
                        )
                        for kc in range(CT):
                            nc.tensor.matmul(
                                pv[:], xTb[kc][:, 128 * t:128 * (t + 1)],
                                wv_sb[:, kc, :],
                                start=False, stop=(kc == CT - 1),
                            )
                        # v = rstd*(raw - mu*csumv) + vb
                        nc.vector.scalar_tensor_tensor(
                            out=vT[:, t, :, 0:HD], in0=pv[:],
                            scalar=stat_cols[:, t:t + 1], in1=vb_bc[:],
                            op0=ALU.mult, op1=ALU.add,
                        )

                    # ---- q,k rows (2 row-tiles x 8 token chunks) ----
                    for mt in range(2):
                        wqk_t = rot1.tile([128, CT, 128], BF16, tag="wqk",
                                          bufs=2, name=f"wqk{mt}")
                        nc.sync.dma_start(out=wqk_t[:], in_=wqk_d[mt])
                        for tc8 in range(TC8):
                            sl = slice(512 * tc8, 512 * (tc8 + 1))
                            pqk = ps1.tile([128, 512], F32, tag="mm", bufs=3,
                                           name=f"pqk{mt}_{tc8}")
                            # fold the -mu*colsum(W) LN correction into the accum
                            nc.tensor.matmul(
                                pqk[:], cqk_sb[0:1, 128 * mt:128 * (mt + 1)],
                                negmu_all[0:1, sl],
                                start=True, stop=False,
                            )
                            for kc in range(CT):
                                nc.tensor.matmul(
                                    pqk[:], wqk_t[:, kc, :], xTb[kc][:, sl],
                                    start=False, stop=(kc == CT - 1),
                                )
                            # zero qkv bias: rstd cancels inside l2-norm
                            sq = rot1.tile([128, 512], BF16, tag="sq", bufs=3,
                                           name=f"sq{mt}_{tc8}")
                            nc.scalar.activation(out=sq[:], in_=pqk[:],
                                                 func=AF.Square)
                            pn = ps1.tile([128, 512], F32, tag="nrm", bufs=2,
                                          name=f"pn{mt}_{tc8}")
                            ones = onesq_sb[:] if mt == 0 else onesk_sb[:]
                            nc.tensor.matmul(pn[:], ones, sq[:], start=True,
                                             stop=True)
                            rrec = rot1.tile([128, 512], F32, tag="rrec",
                                             name=f"rrec{mt}_{tc8}")
                            nc.scalar.activation(out=rrec[:], in_=pn[:],
                                                 func=AF.Abs_reciprocal_sqrt)
                            dst = qTn if mt == 0 else kTn
                            nc.vector.tensor_mul(out=dst[:, sl], in0=pqk[:],
                                                 in1=rrec[:])

                    # prefetch (data-ready at issue): residual xTf
                    for ct in range(CT):
                        nc.sync.dma_start(out=xTf[ct][:],
                                          in_=xtf_d[128 * ct:128 * (ct + 1), :])

                # ==== phase B: attention (2 local heads, both batches) ====
                with (
                    tc.tile_pool(name="rot2", bufs=2) as rot2,
                    tc.tile_pool(name="ps2", bufs=1, space="PSUM") as ps2,
                ):
                    for bb in range(B):
                        for qc in range(QC):
                            blk = QC * bb + qc
                            qsl = slice(L * bb + 512 * qc, L * bb + 512 * (qc + 1))
                            po = [
                                ps2.tile([HD + 1, T], F32, tag=f"o{h2}", bufs=1,
                                         name=f"po{blk}_{h2}")
                                for h2 in range(2)
                            ]
                            pTs = []
                            for kb in range(KM // 2):
                                psS = ps2.tile([128, 4, T], F32, tag="s", bufs=1,
                                               name=f"psS{blk}_{kb}")
                                for kl in range(2):
                                    km = 2 * kb + kl
                                    ksl = slice(L * bb + 128 * km,
                                                L * bb + 128 * (km + 1))
                                    for h2 in range(2):
                                        nc.tensor.matmul(
                                            psS[:, 2 * kl + h2, :],
                                            kTn[64 * h2:64 * (h2 + 1), ksl],
                                            qTn[64 * h2:64 * (h2 + 1), qsl],
                                            start=True, stop=True,
                                        )
                                pT = rot2.tile([128, 4, T], BF16, tag="pT",
                                               bufs=8, name=f"pT{blk}_{kb}")
                                nc.scalar.activation(out=pT[:], in_=psS[:],
                                                     func=AF.Exp)
                                pTs.append(pT)
                                # software pipeline: PV of the previous pair
                                # issues after this pair's S, so the tensor
                                # queue never stalls behind the exp
                                if kb > 0:
                                    for kl in range(2):
                                        km = 2 * (kb - 1) + kl
                                        for h2 in range(2):
                                            nc.tensor.matmul(
                                                po[h2][:],
                                                vT[:, KM * bb + km, h2, 0:HD + 1],
                                                pTs[kb - 1][:, 2 * kl + h2, :],
                                                start=(kb == 1 and kl == 0),
                                                stop=False,
                                            )
                            for kl in range(2):
                                km = KM - 2 + kl
                                for h2 in range(2):
                                    nc.tensor.matmul(
                                        po[h2][:],
                                        vT[:, KM * bb + km, h2, 0:HD + 1],
                                        pTs[KM // 2 - 1][:, 2 * kl + h2, :],
                                        start=False,
                                        stop=(kl == 1),
                                    )
                            for h2 in range(2):
                                # 1/rs = (1/sqrt(rs))^2; head_scale folded sqrt
                                rs = rot2.tile([1, T], BF16, tag="rs")
                                nc.scalar.activation(out=rs[:],
                                                     in_=po[h2][HD:HD + 1, :],
                                                     func=AF.Abs_reciprocal_sqrt)
                                prr = ps2.tile([HD, T], F32, tag="prr", bufs=2,
                                               name=f"prr{h2}")
                                nc.tensor.matmul(
                                    prr[:],
                                    invhs_sb[0:1, HD * h2:HD * (h2 + 1)], rs[:],
                                    start=True, stop=True,
                                )
                                rep = rot2.tile([HD, T], F32, tag="orec")
                                nc.scalar.activation(out=rep[:], in_=prr[:],
                                                     func=AF.Square)
                                nc.vector.tensor_mul(
                                    out=OT[64 * h2:64 * (h2 + 1), qsl],
                                    in0=po[h2][0:HD, :],
                                    in1=rep[:],
                                )
                            nc.gpsimd.dma_start(
                                out=a2a_in[blk], in_=OT[:, qsl]
                            )
                    nc.gpsimd.collective_compute(
                        "AllToAll", ALU.bypass, replica_groups=groups,
                        ins=[a2a_in.opt()], outs=[a2a_out.opt()],
                    )

            # ==== phase C: out projection (transposed out) + ln_attn + ln2 ====
            with (
                tc.tile_pool(name="pc", bufs=1) as pc,
                tc.tile_pool(name="rot3", bufs=2) as rot3,
                tc.tile_pool(name="ps3", bufs=1, space="PSUM") as ps3,
            ):
                wo_sb = pc.tile([128, CT, C], BF16)
                nc.sync.dma_start(
                    out=wo_sb[:],
                    in_=wo_d[:].rearrange("(k p) m -> p k m", p=128),
                )
                oa = [pc.tile([128, T], BF16, name=f"oa{k}") for k in range(CT)]
                for kc in range(CT):
                    nc.scalar.dma_start(out=oa[kc][:], in_=a2a_out[kc])
                yT = [rot3.tile([128, T], F32, tag="yT", bufs=8, name=f"yT{m}")
                      for m in range(CT)]
                for m in range(CT):
                    py = ps3.tile([128, T], F32, tag="y", bufs=3, name=f"py{m}")
                    for kc in range(CT):
                        nc.tensor.matmul(
                            py[:],
                            wo_sb[:, kc, 128 * m:128 * (m + 1)],
                            oa[kc][:],
                            start=(kc == 0), stop=(kc == CT - 1),
                        )
                    nc.vector.tensor_scalar(out=yT[m][:], in0=py[:],
                                            scalar1=outb_sb[:, m:m + 1],
                                            scalar2=None, op0=ALU.add)
                a2_rep, c2_rep = ln_stats_T(yT, rot3, ps3)
                # uT = xT + (yT - mu)*rstd*g' + b'   (g', b' have ls1 folded)
                for ct in range(CT):
                    t1 = rot3.tile([128, T], F32, tag="ut1", name=f"ut1_{ct}")
                    nc.gpsimd.tensor_mul(out=t1[:], in0=yT[ct][:], in1=a2_rep[:])
                    t2 = rot3.tile([128, T], F32, tag="ut2", name=f"ut2_{ct}")
                    nc.gpsimd.tensor_sub(out=t2[:], in0=t1[:], in1=c2_rep[:])
                    t3 = rot3.tile([128, T], F32, tag="ut3", name=f"ut3_{ct}")
                    nc.vector.tensor_scalar(out=t3[:], in0=t2[:],
                                            scalar1=ga_sb[:, ct:ct + 1],
                                            scalar2=ba_sb[:, ct:ct + 1],
                                            op0=ALU.mult, op1=ALU.add)
                    nc.vector.tensor_add(out=uT[ct][:], in0=t3[:], in1=xTf[ct][:])
                a3_rep, c3_rep = ln_stats_T(uT, rot3, ps3)
                for ct in range(CT):
                    # h1T = (uT)*rstd - mu*rstd  (ln2 affine folded into fc)
                    t4 = rot3.tile([128, T], F32, tag="ut4", name=f"ut4_{ct}")
                    eng = nc.gpsimd if ct % 2 == 0 else nc.vector
                    eng.tensor_mul(out=t4[:], in0=uT[ct][:], in1=a3_rep[:])
                    with nc.allow_low_precision(reason="fp8 mlp activations"):
                        eng.tensor_sub(out=h1T[:, ct, :], in0=t4[:],
                                       in1=c3_rep[:])

            # ---- phase D: fc + gelu; proj (transposed) + residual ----
            with (
                tc.tile_pool(name="pd", bufs=1) as pd,
                tc.tile_pool(name="rot4", bufs=2) as rot4,
                tc.tile_pool(name="ps4", bufs=1, space="PSUM") as ps4,
            ):
                gT = pd.tile([128, MT_FC, T], FP8, name="gT")
                for mt in range(MT_FC):
                    wfc_t = rot4.tile([128, 4, 2, 128], FP8, tag="wfc", bufs=6,
                                      name=f"wfc{mt}")
                    nc.sync.dma_start(out=wfc_t[:], in_=wfc_d[mt])
                    pfc = ps4.tile([128, T], F32, tag="fc", bufs=3, name=f"pfc{mt}")
                    for i in range(4):
                        nc.tensor.matmul(
                            pfc[:], wfc_t[:, i, :, :], h1T[:, 2 * i:2 * i + 2, :],
                            start=(i == 0), stop=(i == 3),
                            perf_mode=PM_DR,
                        )
                    nc.scalar.activation(out=gT[:, mt, :], in_=pfc[:],
                                         func=AF.Gelu, scale=1.0 / FC_SCALE,
                                         bias=fcb_sb[:, mt:mt + 1])

                for half in range(2):
                    ppj = [
                        ps4.tile([128, T], F32, tag=f"pj{i}", bufs=1,
                                 name=f"ppj{i}")
                        for i in range(4)
                    ]
                    for pc in range(16):
                        wpj_t = rot4.tile([128, 2, 512], FP8, tag="wpj", bufs=8,
                                          name=f"wpj{half}_{pc}")
                        nc.sync.dma_start(
                            out=wpj_t[:],
                            in_=wpj_d[pc, :, :, 512 * half:512 * (half + 1)],
                        )
                        for i in range(4):
                            nc.tensor.matmul(
                                ppj[i][:],
                                wpj_t[:, :, 128 * i:128 * (i + 1)],
                                gT[:, 2 * pc:2 * pc + 2, :],
                                start=(pc == 0), stop=(pc == 15),
                                perf_mode=PM_DR,
                            )
                    for i in range(4):
                        ct = 4 * half + i
                        o1 = rot4.tile([128, T], F32, tag="ofin",
                                       name=f"of{half}_{i}")
                        nc.vector.tensor_scalar(out=o1[:], in0=ppj[i][:],
                                                scalar1=ls2_sb[:, ct:ct + 1],
                                                scalar2=pjb_sb[:, ct:ct + 1],
                                                op0=ALU.mult, op1=ALU.add)
                        nc.vector.tensor_add(out=o1[:], in0=o1[:], in1=uT[ct][:])
                        nc.gpsimd.dma_start(
                            out=out_d[128 * ct:128 * (ct + 1), :], in_=o1[:]
                        )

    nc.compile()
    return nc


def _host_prep(inp):
    """Shared + per-core (head-pair) folded weights."""
    f32 = np.float32
    ln1_g = np.asarray(inp["ln1_g"], f32)
    ln1_b = np.asarray(inp["ln1_b"], f32)
    ln2_g = np.asarray(inp["ln2_g"], f32)
    ln2_b = np.asarray(inp["ln2_b"], f32)
    in_w = np.asarray(inp["in_proj_w"], f32)
    in_b = np.asarray(inp["in_proj_b"], f32)
    fc_w = np.asarray(inp["fc_w"], f32)
    proj_w = np.asarray(inp["proj_w"], f32)
    ls1 = np.asarray(inp["ls1"], f32)
    ls2 = np.asarray(inp["ls2"], f32)

    # fast path requires zero qkv bias (rstd cancels inside the l2-norm)
    qk_bias = in_b[:2 * C] + ln1_b @ in_w[:2 * C].T
    assert np.abs(qk_bias).max() < 1e-6, "nonzero qk bias: fast path invalid"

    s = {}
    s["woT"] = np.ascontiguousarray(np.asarray(inp["out_w"], f32).T).astype(BF_NP)
    s["outb_s"] = np.ascontiguousarray(
        np.asarray(inp["out_b"], f32).reshape(8, 128).T
    )
    wfcT = (fc_w * ln2_g[None, :]).T * FC_SCALE             # [C, MLP]
    wfc8 = np.clip(wfcT, -240, 240).astype(F8_NP)
    s["wfcT"] = np.ascontiguousarray(
        wfc8.reshape(4, 2, 128, MT_FC, 128).transpose(3, 2, 0, 1, 4)
    )
    fcb = np.asarray(inp["fc_b"], f32) + ln2_b @ fc_w.T
    s["fcb"] = np.ascontiguousarray(fcb.reshape(32, 128).T).astype(f32)
    wpjT = proj_w.T * PJ_SCALE                              # [MLP, C]
    wpj8 = np.clip(wpjT, -240, 240).astype(F8_NP)
    s["wprojT"] = np.ascontiguousarray(
        wpj8.reshape(16, 2, 128, C).transpose(0, 2, 1, 3)
    )
    s["ls2_s"] = np.ascontiguousarray(
        (ls2 / PJ_SCALE).reshape(8, 128).T
    ).astype(f32)
    s["projb_s"] = np.ascontiguousarray(
        (ls2 * np.asarray(inp["proj_b"], f32)).reshape(8, 128).T
    )
    s["gattn_s"] = np.ascontiguousarray(
        (ls1 * np.asarray(inp["ln_attn_g"], f32)).reshape(8, 128).T
    )
    s["battn_s"] = np.ascontiguousarray(
        (ls1 * np.asarray(inp["ln_attn_b"], f32)).reshape(8, 128).T
    )
    onesk = np.zeros((128, 128), f32)
    onesk[:64, :64] = 1.0
    onesk[64:, 64:] = 1.0
    s["onesk"] = onesk.astype(BF_NP)

    lsc = np.exp(np.minimum(np.asarray(inp["logit_scale"], f32).reshape(H),
                            np.log(100.0)))
    hs = np.asarray(inp["head_scale"], f32)

    per_c = []
    for c in range(N_CORES):
        p = {}
        rq = slice(128 * c, 128 * (c + 1))
        rk = slice(C + 128 * c, C + 128 * (c + 1))
        rv = slice(2 * C + 128 * c, 2 * C + 128 * (c + 1))
        w_qk = np.concatenate([in_w[rq], in_w[rk]], 0)        # [256, C]
        wqkT = np.ascontiguousarray((w_qk * ln1_g[None, :]).T).astype(BF_NP)
        p["wqkT"] = np.ascontiguousarray(
            wqkT.reshape(CT, 128, 2, 128).transpose(2, 1, 0, 3)
        )
        p["csumqk"] = wqkT.astype(f32).sum(0).reshape(1, 256).astype(BF_NP)
        w_v = in_w[rv]
        wvT = np.ascontiguousarray((w_v * ln1_g[None, :]).T).astype(BF_NP)
        p["wvT"] = wvT
        p["csumv"] = wvT.astype(f32).sum(0).reshape(1, 128).astype(BF_NP)
        p["vb"] = (in_b[rv] + ln1_b @ w_v.T).reshape(1, 128).astype(f32)
        onesq = np.zeros((128, 128), f32)
        for h2 in range(2):
            h = 2 * c + h2
            onesq[64 * h2:64 * (h2 + 1),
                  64 * h2:64 * (h2 + 1)] = 1.0 / lsc[h] ** 2
        p["onesq"] = onesq.astype(BF_NP)
        invhs = np.zeros((1, 128), f32)
        for h2 in range(2):
            invhs[0, HD * h2:HD * (h2 + 1)] = np.sqrt(hs[2 * c + h2])
        p["invhs"] = invhs.astype(BF_NP)
        per_c.append(p)
    return s, per_c


def kernel(**inputs) -> np.ndarray:
    global _NC_CACHE, LAST_EXEC_NS, LAST_RESULTS
    if _NC_CACHE is None:
        _NC_CACHE = _build()
    nc = _NC_CACHE

    shared, per_c = _host_prep(inputs)
    x = np.asarray(inputs["x"], np.float32)
    # both batches, channel-major: [C, 4096] = [x0^T | x1^T]
    xT = np.ascontiguousarray(
        np.concatenate([x[b].T for b in range(B)], axis=1)
    )
    xTb = xT.astype(BF_NP)
    xTt = np.ascontiguousarray(xTb.T)                          # [4096, C]

    in_maps = []
    for c in range(N_CORES):
        m = dict(shared)
        m.update(per_c[c])
        m["xTb"] = xTb
        m["xTt"] = xTt
        m["xTf"] = np.ascontiguousarray(xT[:, T * c:T * (c + 1)])
        in_maps.append(m)

    kwargs = {}
    if TRACE:
        import os
        os.makedirs(TRACE_DIR, exist_ok=True)
        kwargs = dict(trace=True, tmpdir=TRACE_DIR)
    res = run_bass_kernel_spmd(nc, in_maps, list(range(N_CORES)), **kwargs)
    LAST_EXEC_NS = res.exec_time_ns
    LAST_RESULTS = res
    out = np.zeros((B, L, C), np.float32)
    for c in range(N_CORES):
        b, r = c // 4, c % 4
        out[b, L // 4 * r:L // 4 * (r + 1), :] = res.results[c]["outT"].T
    return out
